# revision 1
# baseline (speedup 1.0000x reference)
"""BlockGlobalAttentionProduct Trainium2 kernel.

Sharding: 24 (n,h) pairs across 8 cores, 3 per core. Each core, per (n,h):
  - dma_gather of interleaved [K|V] bf16 rows (256B) by local_idx / global_idx
  - PE transposes build K^T (d on partitions) for the score matmuls
  - scores^T computed per key tile (keys on partitions, queries on free dim)
  - exp on ScalarE (scale=1/8 folded in); window padding masked by zeroing
  - PV accumulated in ctx^T form (d+1 rows incl. sum-of-exp) in PSUM
  - host does final divide-by-denominator + transpose during unshard
"""

import sys

sys.path.insert(0, "/opt/trn_rl_repo")

import numpy as np
import ml_dtypes

import concourse.bacc as bacc
import concourse.mybir as mybir
from concourse import bass, tile, bass_utils, library_config

# problem shape (hardcoded per spec)
N, H, T, D = 2, 12, 4096, 64
NH = N * H            # 24
NCORES = 8
PER_CORE = NH // NCORES  # 3
NTILE = T // 128      # 32 key tiles per table
NSEG = 8              # query segments of 512
QH_W = 128 + T + 256  # qT halo width: cols [-128, 4352)
NEG0 = 0

BF16 = mybir.dt.bfloat16
F32 = mybir.dt.float32
I16 = mybir.dt.int16


def _intervals(a0, width, s):
    """Pieces of window [a0, a0+width) mod T intersected with segment
    [512s, 512(s+1)). Yields (tile_col_offset, seg_col_offset, length)."""
    lo, hi = 512 * s, 512 * (s + 1)
    pieces = []
    a0 %= T
    if a0 + width <= T:
        pieces.append((a0, a0 + width, 0))
    else:
        pieces.append((a0, T, 0))
        pieces.append((0, (a0 + width) % T, T - a0))
    out = []
    for wa, wb, base in pieces:
        u, v = max(wa, lo), min(wb, hi)
        if u < v:
            out.append((base + (u - wa), u - lo, v - u))
    return out


def build_program():
    nc = bacc.Bacc("TRN2", target_bir_lowering=False, debug=False,
                   num_devices=NCORES)

    qTh = nc.dram_tensor("qTh", [PER_CORE, 64, QH_W], BF16, kind="ExternalInput")
    kvT = nc.dram_tensor("kv", [PER_CORE, T, 128], BF16, kind="ExternalInput")
    gkT_d = nc.dram_tensor("gkT", [PER_CORE, 64, 64], BF16, kind="ExternalInput")
    # gv1[:, :, p, :]: [gv|1] rows zero-padded on the opposite 64-partition
    # half, so gtok PV can contract the full 128 partitions of the
    # column-paired expT layout (parity p selects which half is live).
    gv1_d = nc.dram_tensor("gv1", [PER_CORE, 128, 2, 65], BF16, kind="ExternalInput")
    lidx_d = nc.dram_tensor("lidx", [PER_CORE, 128, 256], I16, kind="ExternalInput")
    gidx_d = nc.dram_tensor("gidx", [PER_CORE, 128, 256], I16, kind="ExternalInput")
    ident_d = nc.dram_tensor("ident", [128, 128], BF16, kind="ExternalInput")
    out_d = nc.dram_tensor("ctxT", [PER_CORE, 65, T], F32, kind="ExternalOutput")

    EXP = mybir.ActivationFunctionType.Exp

    with tile.TileContext(nc) as tc:
        with (
            tc.tile_pool(name="const", bufs=1) as constp,
            tc.tile_pool(name="land", bufs=2) as land,
            tc.tile_pool(name="work", bufs=1) as work,
            tc.tile_pool(name="outp", bufs=2) as outp,
            tc.tile_pool(name="ps1", bufs=2, space="PSUM") as ps1,
            tc.tile_pool(name="psL", bufs=1, space="PSUM") as psL,
            tc.tile_pool(name="psG", bufs=1, space="PSUM") as psG,
        ):
            ident = constp.tile([128, 128], BF16, tag="ident")
            nc.sync.dma_start(ident[:], ident_d[:])
            lib_i = nc.gpsimd.load_library(library_config.mlp)

            for i in range(PER_CORE):
                # ---------------- loads + gathers ----------------
                q_sb = land.tile([64, QH_W], BF16, tag="q")
                kvL = land.tile([128, NTILE, 128], BF16, tag="kvL")
                kvG = land.tile([128, NTILE, 128], BF16, tag="kvG")
                li_sb = land.tile([128, 256], I16, tag="li")
                gi_sb = land.tile([128, 256], I16, tag="gi")
                gkT = land.tile([64, 64], BF16, tag="gkT")
                gv1 = land.tile([128, 2, 65], BF16, tag="gv1")

                nc.sync.dma_start(q_sb[:], qTh[i])
                nc.sync.dma_start(gkT[:], gkT_d[i])
                nc.sync.dma_start(gv1[:], gv1_d[i])
                nc.gpsimd.dma_start(li_sb[:], lidx_d[i])
                nc.gpsimd.dma_start(gi_sb[:], gidx_d[i])
                g1 = nc.gpsimd.dma_gather(kvL[:], kvT[i], li_sb[:], T, T, 128,
                                          single_packet=False)
                g2 = nc.gpsimd.dma_gather(kvG[:], kvT[i], gi_sb[:], T, T, 128,
                                          single_packet=False)
                if i == 0:
                    from concourse.tile_rust import add_dep_helper
                    add_dep_helper(lib_i.ins, g1.ins, reason="lib before gather")

                # ---------------- K^T construction ----------------
                # all K^T tiles live on partitions [0,64) — the PE on this
                # runtime rejects row-group (contraction base) alternation,
                # so every score matmul contracts at base partition 0.
                klT = work.tile([64, 4096], BF16, tag="klT")
                kgT = work.tile([64, 4096], BF16, tag="kgT")
                for kv_sb, kT in ((kvL, klT), (kvG, kgT)):
                    for grp in range(4):         # 8 tiles per psum pack
                        tp = ps1.tile([64, 1024], BF16, tag="b1")
                        for pp in range(8):
                            c = grp * 8 + pp
                            nc.tensor.transpose(
                                out=tp[:, pp * 128:(pp + 1) * 128],
                                in_=kv_sb[:, c, 0:64], identity=ident[:])
                        nc.vector.tensor_copy(
                            kT[:, grp * 1024:(grp + 1) * 1024], tp[:])

                # ---------------- V1 = [V | 1] ----------------
                v1L = work.tile([128, NTILE, 65], BF16, tag="v1L")
                v1G = work.tile([128, NTILE, 65], BF16, tag="v1G")
                for kv_sb, v1 in ((kvL, v1L), (kvG, v1G)):
                    nc.gpsimd.memset(v1[:, :, 64:65], 1.0)
                    nc.vector.tensor_copy(v1[:, :, 0:64], kv_sb[:, :, 64:128])

                # ---------------- scores^T + exp ----------------
                expL = work.tile([128, NTILE, 256], BF16, tag="expL")
                expG = work.tile([128, NTILE, 384], BF16, tag="expG")
                expT = work.tile([128, 4, 512], BF16, tag="expT")

                # local: per key tile c, queries [(2c-1)*64, (2c+3)*64)
                # two col-group matmuls per tile (key halves at output
                # partition halves) — contraction base 0 for both.
                for p in range(8):               # packs of 4 tiles
                    st = psL.tile([128, 1024], F32, tag="pL")
                    for j in range(4):
                        c = 4 * p + j
                        rhs = q_sb[:, 64 + 128 * c:64 + 128 * c + 256]
                        nc.tensor.matmul(st[:, j * 256:(j + 1) * 256],
                                         klT[:, 128 * c:128 * c + 128], rhs,
                                         start=True, stop=True)
                    nc.scalar.activation(expL[:, 4 * p:4 * p + 4, :],
                                         st[:].rearrange("p (a b) -> p a b", b=256),
                                         EXP, scale=0.125)
                    for j in range(4):
                        c = 4 * p + j
                        nc.gpsimd.memset(expL[64:128, c, 0:64], NEG0)
                        nc.gpsimd.memset(expL[0:64, c, 192:256], NEG0)

                # global: per key tile t, queries [(t-1)*128, (t+2)*128)
                for p in range(8):
                    st = psG.tile([128, 2048], F32, tag="pG")
                    for j in range(4):
                        t = 4 * p + j
                        rhs = q_sb[:, 128 * t:128 * t + 384]
                        nc.tensor.matmul(st[:, j * 512:j * 512 + 384],
                                         kgT[:, 128 * t:128 * t + 128], rhs,
                                         start=True, stop=True)
                    src = st[:].rearrange("p (a b) -> p a b", b=512)[:, :, 0:384]
                    nc.scalar.activation(expG[:, 4 * p:4 * p + 4, :], src,
                                         EXP, scale=0.125)

                # gtok: per query block g of 512
                for p in range(4):
                    st = ps1.tile([128, 512], F32, tag="b1")
                    for j in range(2):
                        g = 2 * p + j
                        nc.tensor.matmul(
                            st[j * 64:j * 64 + 64, 0:512],
                            gkT[:], q_sb[:, 128 + 512 * g:128 + 512 * g + 512],
                            start=True, stop=True,
                            tile_position=(0, j * 64))
                    nc.scalar.activation(expT[:, p, :], st[:], EXP, scale=0.125)

                # ---------------- PV (ctx^T accumulate) ----------------
                ctx_sb = outp.tile([65, T], F32, tag="ctx")
                for s in range(NSEG):
                    acc = ps1.tile([65, 512], F32, tag="b1")
                    mms = []
                    # gtok initializes the whole segment (full-128 contraction;
                    # the inactive parity half of gv1 is zero)
                    mms.append((gv1[:, s % 2, :], expT[:, s // 2, 0:512], 0, 512))
                    for c in range(NTILE):
                        for (tcol, scol, ln) in _intervals((2 * c - 1) * 64, 256, s):
                            mms.append((v1L[:, c, :],
                                        expL[:, c, tcol:tcol + ln], scol, ln))
                    for t in range(NTILE):
                        for (tcol, scol, ln) in _intervals((t - 1) * 128, 384, s):
                            mms.append((v1G[:, t, :],
                                        expG[:, t, tcol:tcol + ln], scol, ln))
                    for mi, (lhsT, rhs, scol, ln) in enumerate(mms):
                        nc.tensor.matmul(acc[:, scol:scol + ln], lhsT, rhs,
                                         start=(mi == 0), stop=(mi == len(mms) - 1),
                                         skip_group_check=True)
                    nc.vector.tensor_copy(ctx_sb[:, 512 * s:512 * (s + 1)], acc[:])

                nc.sync.dma_start(out_d[i], ctx_sb[:])

    nc.compile()
    return nc


_CACHED = None


def _get_program():
    global _CACHED
    if _CACHED is None:
        _CACHED = build_program()
    return _CACHED


def _prep_core_inputs(q, k, v, gk, gv, lidx, gidx, pairs):
    """Build one core's input dict for its list of (n,h) pairs."""
    bf = ml_dtypes.bfloat16
    qTh = np.empty((PER_CORE, 64, QH_W), dtype=bf)
    kv = np.empty((PER_CORE, T, 128), dtype=bf)
    gkT = np.empty((PER_CORE, 64, 64), dtype=bf)
    gv1 = np.zeros((PER_CORE, 128, 2, 65), dtype=bf)
    li = np.empty((PER_CORE, 128, 256), dtype=np.int16)
    gi = np.empty((PER_CORE, 128, 256), dtype=np.int16)
    for s, (n, h) in enumerate(pairs):
        qt = np.ascontiguousarray(q[n, h].T)            # (64, T) f32
        qth = np.concatenate([qt[:, T - 128:], qt, qt[:, :256]], axis=1)
        qTh[s] = qth.astype(bf)
        kv[s, :, 0:64] = k[n, h].astype(bf)
        kv[s, :, 64:128] = v[n, h].astype(bf)
        gkT[s] = np.ascontiguousarray(gk[n, h].T).astype(bf)
        g1 = np.concatenate([gv[n, h], np.ones((64, 1), np.float32)],
                            axis=1).astype(bf)
        gv1[s, 0:64, 0] = g1      # parity 0: top half live
        gv1[s, 64:128, 1] = g1    # parity 1: bottom half live
        for arr, src in ((li, lidx), (gi, gidx)):
            ix = src[n, h, :, 0].astype(np.int16)       # (T,)
            arr[s] = np.tile(ix.reshape(T // 16, 16).T, (8, 1))
    ident = np.eye(128, dtype=bf)
    return {"qTh": qTh, "kv": kv, "gkT": gkT, "gv1": gv1,
            "lidx": li, "gidx": gi, "ident": ident}


def kernel(query_layer, key_layer, value_layer, attention_mask, local_idx,
           global_idx, global_key, global_value, global_mask):
    # attention_mask / global_mask are all-zero in this problem's input spec;
    # they contribute nothing to the scores and are not shipped to the device.
    q = np.asarray(query_layer, np.float32)
    k = np.asarray(key_layer, np.float32)
    v = np.asarray(value_layer, np.float32)
    gk = np.asarray(global_key, np.float32)
    gv = np.asarray(global_value, np.float32)
    li = np.asarray(local_idx)
    gi = np.asarray(global_idx)

    nc = _get_program()
    in_maps = []
    for m in range(NCORES):
        pairs = [((3 * m + s) // H, (3 * m + s) % H) for s in range(PER_CORE)]
        in_maps.append(_prep_core_inputs(q, k, v, gk, gv, li, gi, pairs))
    res = bass_utils.run_bass_kernel_spmd(nc, in_maps, core_ids=list(range(NCORES)))

    out = np.empty((N, H, T, D), np.float32)
    for m in range(NCORES):
        ctxT = res.results[m]["ctxT"]                   # (3, 65, T)
        for s in range(PER_CORE):
            n, h = (3 * m + s) // H, (3 * m + s) % H
            out[n, h] = (ctxT[s, :64] / ctxT[s, 64:65]).T
    return out

